# revision 14
# baseline (speedup 1.0000x reference)
"""GCN link-predictor kernel for 8 Trainium2 NeuronCores (Bass/Tile).

Strategy (SPMD, single program on 8 cores, no core-dependent addressing):
  - Host: append self loops, sort edges by dst, partition nodes into 8
    contiguous ranges (12500/core, padded to 12544 = 98 tiles of 128).
    Core q owns all edges whose dst lies in its range, grouped per
    128-node destination tile, padded to a uniform K chunks of 128 edges.
  - deg:   per-chunk one-hot matrices W[e, dst_local] = ew[e] (built on DVE
           from an iota constant via fused is_equal*mult tensor_scalar),
           deg_tile = sum_k W_k^T @ 1.  dinv = 1/sqrt(max(deg,1)).
           AllGather dinv shards -> full dinv table in SBUF.
  - layer: h' = dinv * (x @ W)  (full table per core; lhsT = host-side
           pre-transposed x tiles so the GEMM emits node-major tiles),
           stored bf16 in DRAM.  Aggregation per owned dst tile:
           indirect-DMA gather of 128 h' rows per chunk + one-hot matmul
           accumulated in PSUM; evict relu(dinv*psum + b).
           Layer-1 output is PE-transposed and AllGathered as [1024,12544]
           so layer-2 GEMM can slice lhsT tiles directly; layer-2 output is
           AllGathered node-major for the label gathers.
  - labels: gather out2[el0], out2[el1], res = sum(a*b*w_vec) + sum(lin_b)
           where w_vec = lin_W @ 1 (the final linear collapses to a
           weighted inner product).
"""

import os
import sys

import numpy as np

for _p in ("/opt/trn_rl_repo",):
    if _p not in sys.path:
        sys.path.insert(0, _p)

import ml_dtypes  # noqa: E402

import concourse.bacc as bacc  # noqa: E402
import concourse.bass as bass  # noqa: E402
import concourse.mybir as mybir  # noqa: E402
from concourse.bass import IndirectOffsetOnAxis  # noqa: E402
from concourse.bass_utils import run_bass_kernel_spmd  # noqa: E402
from concourse.tile import TileContext  # noqa: E402

P = 128
NC = 8
BF = mybir.dt.bfloat16
F32 = mybir.dt.float32
I32 = mybir.dt.int32

LAST_EXEC_NS = None
LAST_RESULTS = None


class Cfg:
    def __init__(self, n_nodes, n_labels):
        assert n_nodes % NC == 0
        self.n_nodes = n_nodes
        self.nodes_per_core = n_nodes // NC
        self.tiles_per_core = -(-self.nodes_per_core // P)
        self.n_loc = self.tiles_per_core * P
        self.n_pad = NC * self.n_loc
        self.n_labels = n_labels
        self.lab_per_core = -(-n_labels // NC)
        self.lab_chunks = -(-self.lab_per_core // P)


FULL = Cfg(100000, 200000)


# ---------------------------------------------------------------- host prep


def _pad_ids(cfg, ids):
    q, l = np.divmod(ids, cfg.nodes_per_core)
    q = np.minimum(q, NC - 1)
    l = ids - q * cfg.nodes_per_core
    return q * cfg.n_loc + l, q, l


def preprocess(cfg, x, edge_index, edge_weight, edge_label_index):
    n = cfg.n_nodes
    src = np.concatenate([edge_index[0], np.arange(n)]).astype(np.int64)
    dst = np.concatenate([edge_index[1], np.arange(n)]).astype(np.int64)
    ew = np.concatenate(
        [edge_weight.astype(np.float32), np.ones(n, np.float32)]
    )

    src_pad, _, _ = _pad_ids(cfg, src)
    _, dq, dl = _pad_ids(cfg, dst)
    T = cfg.tiles_per_core
    bucket = dq * T + dl // P
    counts = np.bincount(bucket, minlength=NC * T).reshape(NC, T)
    # per-tile chunk count: max over cores (keeps the SPMD program uniform
    # while minimizing total chunks; walrus caps indirect DMAs at ~4096)
    K_arr = np.maximum(1, -(-counts.max(axis=0) // P))  # [T]
    off = np.zeros(T + 1, np.int64)
    off[1:] = np.cumsum(K_arr)
    C = int(off[-1])

    order = np.argsort(bucket, kind="stable")
    sb = bucket[order]
    starts = np.zeros(NC * T + 1, np.int64)
    starts[1:] = np.cumsum(counts.reshape(-1))
    pos = np.arange(len(order)) - starts[sb]
    dest_core = sb // T
    dest_flat = off[sb % T] * P + pos  # within-core flat edge slot

    srci = np.zeros((NC, C * P), np.int32)
    dstl = np.zeros((NC, C * P), np.float32)
    ewp = np.zeros((NC, C * P), np.float32)
    srci[dest_core, dest_flat] = src_pad[order]
    dstl[dest_core, dest_flat] = (dl % P)[order]
    ewp[dest_core, dest_flat] = ew[order]

    # [core, C*P] -> [core, 128, C]   (partition = edge slot within chunk)
    def to_pc(a, dt):
        return np.ascontiguousarray(
            a.reshape(NC, C, P).transpose(0, 2, 1)
        ).astype(dt)

    n_gather = 2 * C + 2 * cfg.lab_chunks
    assert n_gather <= 4080, f"indirect DMA budget exceeded: {n_gather}"


    srci = to_pc(srci, np.int32)
    meta = np.concatenate(
        [to_pc(dstl, np.float32), to_pc(ewp, np.float32)], axis=-1
    ).astype(ml_dtypes.bfloat16)

    # labels
    el_pad, _, _ = _pad_ids(cfg, edge_label_index.astype(np.int64))
    LC = cfg.lab_chunks
    el0 = np.zeros((NC, LC * P), np.int32)
    el1 = np.zeros((NC, LC * P), np.int32)
    lpc = cfg.lab_per_core
    for q in range(NC):
        lo, hi = q * lpc, min((q + 1) * lpc, cfg.n_labels)
        el0[q, : hi - lo] = el_pad[0, lo:hi]
        el1[q, : hi - lo] = el_pad[1, lo:hi]
    el0 = np.ascontiguousarray(el0.reshape(NC, LC, P).transpose(0, 2, 1))
    el1 = np.ascontiguousarray(el1.reshape(NC, LC, P).transpose(0, 2, 1))

    # node features, padded + transposed
    pid_all, _, _ = _pad_ids(cfg, np.arange(n))
    x_pad = np.zeros((cfg.n_pad, P), np.float32)
    x_pad[pid_all] = x
    xT = np.ascontiguousarray(x_pad.T).astype(ml_dtypes.bfloat16)

    kmax = int(K_arr.max())
    iota_rep = np.tile(
        np.arange(P, dtype=np.float32)[None, :], (P, kmax)
    ).astype(ml_dtypes.bfloat16)
    return dict(srci=srci, meta=meta, el0=el0, el1=el1, xT=xT,
                K_arr=[int(v) for v in K_arr], iota_rep=iota_rep)


# ------------------------------------------------------------- bass program


def build_program(cfg, K_arr, linb_sum, phase=99):
    K_off = [0]
    for v in K_arr:
        K_off.append(K_off[-1] + v)
    KMAX = max(K_arr)
    T = cfg.tiles_per_core
    C = K_off[-1]
    NPAD, NLOC, LC = cfg.n_pad, cfg.n_loc, cfg.lab_chunks
    GT = NC * T  # global tiles
    rg = [list(range(NC))]

    nc = bacc.Bacc(None, target_bir_lowering=False, debug=False)

    xT = nc.declare_dram_parameter("xT", [P, NPAD], BF, False)
    srci_d = nc.declare_dram_parameter("srci", [P, C], I32, False)
    meta_d = nc.declare_dram_parameter("meta", [P, 2 * C], BF, False)
    el0_d = nc.declare_dram_parameter("el0", [P, LC], I32, False)
    el1_d = nc.declare_dram_parameter("el1", [P, LC], I32, False)
    iota_d = nc.declare_dram_parameter("iota", [P, KMAX * P], BF, False)
    ident_d = nc.declare_dram_parameter("ident", [P, P], BF, False)
    w1_d = nc.declare_dram_parameter("w1", [P, P], BF, False)
    w2_d = nc.declare_dram_parameter("w2", [P, P], BF, False)
    b1_d = nc.declare_dram_parameter("b1bc", [P, P], F32, False)
    b2_d = nc.declare_dram_parameter("b2bc", [P, P], F32, False)
    wv_d = nc.declare_dram_parameter("wvbc", [P, P], F32, False)
    res_d = nc.declare_dram_parameter("res", [P, LC], F32, True)

    htab = nc.dram_tensor("htab", [NPAD, P], BF)
    dinv_sh = nc.dram_tensor("dinv_sh", [1, NLOC], F32)
    dinv_ag = nc.dram_tensor("dinv_ag", [NC, NLOC], F32, addr_space="Shared")
    o1t_sh = nc.dram_tensor("o1t_sh", [P, NLOC], BF)
    o1t_ag = nc.dram_tensor("o1t_ag", [NC * P, NLOC], BF, addr_space="Shared")
    o2_sh = nc.dram_tensor("o2_sh", [NLOC, P], BF)
    o2_ag = nc.dram_tensor("o2_ag", [NPAD, P], BF)

    AF = mybir.ActivationFunctionType
    OP = mybir.AluOpType

    with TileContext(nc) as tc:
        with (
            tc.tile_pool(name="const", bufs=1) as cp,
            tc.tile_pool(name="wtile", bufs=6) as wp,
            tc.tile_pool(name="htile", bufs=8) as hp,
            tc.tile_pool(name="gemm", bufs=6) as gp,
            tc.tile_pool(name="evict", bufs=4) as ep,
            tc.tile_pool(name="lab", bufs=8) as lp,
            tc.tile_pool(name="ps_deg", bufs=2, space="PSUM") as psd,
            tc.tile_pool(name="ps_gemm", bufs=2, space="PSUM") as psg,
            tc.tile_pool(name="ps_agg", bufs=2, space="PSUM") as psa,
            tc.tile_pool(name="ps_tr", bufs=2, space="PSUM") as pst,
        ):
            # ---- persistent SBUF ----
            srci_sb = cp.tile([P, C], I32)
            nc.sync.dma_start(out=srci_sb[:], in_=srci_d[:, :])
            meta_sb = cp.tile([P, 2 * C], BF)
            nc.sync.dma_start(out=meta_sb[:], in_=meta_d[:, :])
            el0_sb = cp.tile([P, LC], I32)
            nc.sync.dma_start(out=el0_sb[:], in_=el0_d[:, :])
            el1_sb = cp.tile([P, LC], I32)
            nc.sync.dma_start(out=el1_sb[:], in_=el1_d[:, :])
            iota_sb = cp.tile([P, KMAX * P], BF)
            nc.sync.dma_start(out=iota_sb[:], in_=iota_d[:, :])
            ident_sb = cp.tile([P, P], BF)
            nc.sync.dma_start(out=ident_sb[:], in_=ident_d[:, :])
            w1_sb = cp.tile([P, P], BF)
            nc.sync.dma_start(out=w1_sb[:], in_=w1_d[:, :])
            w2_sb = cp.tile([P, P], BF)
            nc.sync.dma_start(out=w2_sb[:], in_=w2_d[:, :])
            b1_sb = cp.tile([P, P], F32)
            nc.sync.dma_start(out=b1_sb[:], in_=b1_d[:, :])
            b2_sb = cp.tile([P, P], F32)
            nc.sync.dma_start(out=b2_sb[:], in_=b2_d[:, :])
            wv_sb = cp.tile([P, P], F32)
            nc.sync.dma_start(out=wv_sb[:], in_=wv_d[:, :])
            ones_sb = cp.tile([P, 1], BF)
            nc.vector.memset(ones_sb[:], 1.0)
            deg_sb = cp.tile([P, T], F32)
            dinv_own = cp.tile([P, T], F32)
            dinvF = cp.tile([P, GT], F32)
            res_sb = cp.tile([P, LC], F32)

            iota3 = iota_sb[:].rearrange("p (g e) -> p g e", e=P)

            def build_w(lt):
                # one-hot W for all chunks of tile lt in two batched ops
                K = K_arr[lt]
                c0 = K_off[lt]
                w = wp.tile([P, KMAX * P], BF, tag="w")
                w3 = w[:, : K * P].rearrange("p (g e) -> p g e", e=P)
                nc.vector.tensor_tensor(
                    out=w3,
                    in0=iota3[:, :K, :],
                    in1=meta_sb[:, c0 : c0 + K].to_broadcast([P, K, P]),
                    op=OP.is_equal,
                )
                nc.vector.tensor_tensor(
                    out=w3,
                    in0=w3,
                    in1=meta_sb[:, C + c0 : C + c0 + K].to_broadcast(
                        [P, K, P]
                    ),
                    op=OP.mult,
                )
                return w

            # ---- deg pass ----
            for lt in range(T):
                pd = psd.tile([P, 1], F32)
                w = build_w(lt)
                K = K_arr[lt]
                for k in range(K):
                    nc.tensor.matmul(
                        out=pd[:],
                        lhsT=w[:, k * P : (k + 1) * P],
                        rhs=ones_sb[:],
                        start=(k == 0),
                        stop=(k == K - 1),
                    )
                nc.scalar.activation(deg_sb[:, lt : lt + 1], pd[:], AF.Copy)
            # dinv = 1/sqrt(max(deg,1));  deg>=1 for real nodes (self loop),
            # dead padding nodes get deg=1 to avoid inf/NaN.
            nc.vector.tensor_scalar_max(deg_sb[:], deg_sb[:], 1.0)
            rec_sb = cp.tile([P, T], F32)
            nc.vector.reciprocal(rec_sb[:], deg_sb[:])
            nc.scalar.activation(dinv_own[:], rec_sb[:], AF.Sqrt)
            nc.sync.dma_start(
                out=dinv_sh.ap().rearrange("a (p l) -> (a p) l", p=P),
                in_=dinv_own[:],
            )
            nc.gpsimd.collective_compute(
                "AllGather",
                OP.bypass,
                replica_groups=rg,
                ins=[dinv_sh[:, :]],
                outs=[dinv_ag[:, :]],
            )
            nc.sync.dma_start(
                out=dinvF[:].rearrange("p (q l) -> p q l", q=NC),
                in_=dinv_ag.ap().rearrange("q (p l) -> p q l", p=P),
            )
            if phase <= 1:
                nc.sync.dma_start(out=res_d[:, :], in_=dinvF[:, :LC])

            # ---- h' table GEMM pass ----
            def gemm_pass(layer):
                w_sb = w1_sb if layer == 1 else w2_sb
                for t in range(GT):
                    lhsT = gp.tile([P, P], BF, tag="lhsT")
                    if layer == 1:
                        nc.sync.dma_start(
                            out=lhsT[:], in_=xT[:, t * P : (t + 1) * P]
                        )
                    else:
                        q, lt = divmod(t, T)
                        nc.sync.dma_start(
                            out=lhsT[:],
                            in_=o1t_ag[
                                q * P : (q + 1) * P, lt * P : (lt + 1) * P
                            ],
                        )
                    pg = psg.tile([P, P], F32)
                    nc.tensor.matmul(
                        out=pg[:], lhsT=lhsT[:], rhs=w_sb[:],
                        start=True, stop=True,
                    )
                    hbf = gp.tile([P, P], BF, tag="hbf")
                    nc.scalar.activation(
                        hbf[:], pg[:], AF.Copy, scale=dinvF[:, t : t + 1]
                    )
                    nc.sync.dma_start(
                        out=htab[t * P : (t + 1) * P, :], in_=hbf[:]
                    )

            # ---- aggregation pass over owned dst tiles ----
            def agg_pass(layer):
                b_sb = b1_sb if layer == 1 else b2_sb
                for lt in range(T):
                    pa = psa.tile([P, P], F32)
                    w = build_w(lt)
                    K = K_arr[lt]
                    for k in range(K):
                        c = K_off[lt] + k
                        h = hp.tile([P, P], BF, tag="h")
                        nc.gpsimd.indirect_dma_start(
                            out=h[:],
                            out_offset=None,
                            in_=htab[:, :],
                            in_offset=IndirectOffsetOnAxis(
                                ap=srci_sb[:, c : c + 1], axis=0
                            ),
                        )
                        nc.tensor.matmul(
                            out=pa[:],
                            lhsT=w[:, k * P : (k + 1) * P],
                            rhs=h[:],
                            start=(k == 0),
                            stop=(k == K - 1),
                        )
                    t1 = ep.tile([P, P], F32, tag="t1")
                    nc.scalar.activation(
                        t1[:], pa[:], AF.Copy,
                        scale=dinv_own[:, lt : lt + 1],
                    )
                    nc.vector.tensor_tensor(
                        out=t1[:], in0=t1[:], in1=b_sb[:], op=OP.add
                    )
                    obf = ep.tile([P, P], BF, tag="obf")
                    nc.scalar.activation(obf[:], t1[:], AF.Relu)
                    if layer == 1:
                        pt = pst.tile([P, P], BF)
                        nc.tensor.transpose(
                            out=pt[:], in_=obf[:], identity=ident_sb[:]
                        )
                        otb = ep.tile([P, P], BF, tag="otb")
                        nc.scalar.activation(otb[:], pt[:], AF.Copy)
                        nc.sync.dma_start(
                            out=o1t_sh[:, lt * P : (lt + 1) * P], in_=otb[:]
                        )
                    else:
                        nc.sync.dma_start(
                            out=o2_sh[lt * P : (lt + 1) * P, :], in_=obf[:]
                        )

            if phase >= 2:
                gemm_pass(1)
            if phase == 2:
                hprobe = cp.tile([P, P], BF)
                nc.sync.dma_start(out=hprobe[:], in_=htab[0:P, :])
                probe_f = cp.tile([P, P], F32)
                nc.vector.tensor_copy(probe_f[:], hprobe[:])
                nc.sync.dma_start(out=res_d[:, :], in_=probe_f[:, :LC])
            if phase >= 3:
                agg_pass(1)
            if phase == 3:
                oprobe = cp.tile([P, P], BF)
                nc.sync.dma_start(out=oprobe[:], in_=o1t_sh[:, 0:P])
                oprobe_f = cp.tile([P, P], F32)
                nc.vector.tensor_copy(oprobe_f[:], oprobe[:])
                nc.sync.dma_start(out=res_d[:, :], in_=oprobe_f[:, :LC])
            if phase >= 4:
                nc.gpsimd.collective_compute(
                    "AllGather",
                    OP.bypass,
                    replica_groups=rg,
                    ins=[o1t_sh[:, :]],
                    outs=[o1t_ag[:, :]],
                )
            if phase == 4:
                oprobe = cp.tile([P, P], BF)
                nc.sync.dma_start(out=oprobe[:], in_=o1t_ag[0:P, 0:P])
                oprobe_f = cp.tile([P, P], F32)
                nc.vector.tensor_copy(oprobe_f[:], oprobe[:])
                nc.sync.dma_start(out=res_d[:, :], in_=oprobe_f[:, :LC])
            if phase >= 5:
                gemm_pass(2)
            if phase >= 5:
                agg_pass(2)
                nc.gpsimd.collective_compute(
                    "AllGather",
                    OP.bypass,
                    replica_groups=rg,
                    ins=[o2_sh[:, :]],
                    outs=[o2_ag[:, :]],
                )

            # ---- label pass ----
            for c in range(LC if phase >= 6 else 0):
                a = lp.tile([P, P], BF, tag="a")
                nc.gpsimd.indirect_dma_start(
                    out=a[:],
                    out_offset=None,
                    in_=o2_ag[:, :],
                    in_offset=IndirectOffsetOnAxis(
                        ap=el0_sb[:, c : c + 1], axis=0
                    ),
                )
                b = lp.tile([P, P], BF, tag="b")
                nc.gpsimd.indirect_dma_start(
                    out=b[:],
                    out_offset=None,
                    in_=o2_ag[:, :],
                    in_offset=IndirectOffsetOnAxis(
                        ap=el1_sb[:, c : c + 1], axis=0
                    ),
                )
                prod = lp.tile([P, P], F32, tag="prod")
                nc.vector.tensor_tensor(
                    out=prod[:], in0=a[:], in1=b[:], op=OP.mult
                )
                scr = lp.tile([P, P], F32, tag="scr")
                nc.vector.tensor_tensor(
                    out=scr[:], in0=prod[:], in1=wv_sb[:], op=OP.mult
                )
                nc.vector.reduce_sum(
                    res_sb[:, c : c + 1], scr[:], axis=mybir.AxisListType.X
                )
            if phase >= 6:
                nc.vector.tensor_scalar_add(
                    res_sb[:], res_sb[:], float(linb_sum)
                )
                nc.sync.dma_start(out=res_d[:, :], in_=res_sb[:])

    nc.finalize()
    return nc


# ------------------------------------------------------------------ driver


def make_in_maps(cfg, prep, W1, b1, W2, b2, lin_W, lin_b):
    consts = dict(
        xT=prep["xT"],
        iota=prep["iota_rep"],
        ident=np.eye(P, dtype=np.float32).astype(ml_dtypes.bfloat16),
        w1=W1.astype(np.float32).astype(ml_dtypes.bfloat16),
        w2=W2.astype(np.float32).astype(ml_dtypes.bfloat16),
        b1bc=np.tile(b1.astype(np.float32)[None, :], (P, 1)),
        b2bc=np.tile(b2.astype(np.float32)[None, :], (P, 1)),
        wvbc=np.tile(
            lin_W.astype(np.float32).sum(axis=1)[None, :], (P, 1)
        ),
    )
    in_maps = []
    for q in range(NC):
        m = dict(consts)
        m.update(
            srci=prep["srci"][q],
            meta=prep["meta"][q],
            el0=prep["el0"][q],
            el1=prep["el1"][q],
        )
        in_maps.append(m)
    return in_maps


def assemble_output(cfg, results):
    outs = []
    for q in range(NC):
        r = np.asarray(results[q]["res"], np.float32)  # [128, LC]
        outs.append(r.T.reshape(-1)[: cfg.lab_per_core])
    return np.concatenate(outs)[: cfg.n_labels].astype(np.float32)


def run(cfg, x, edge_index, edge_weight, edge_label_index,
        W1, b1, W2, b2, lin_W, lin_b, trace=False, phase=99):
    global LAST_EXEC_NS, LAST_RESULTS
    prep = preprocess(cfg, np.asarray(x), np.asarray(edge_index),
                      np.asarray(edge_weight), np.asarray(edge_label_index))
    linb_sum = float(np.asarray(lin_b, np.float64).sum())
    nc = build_program(cfg, prep["K_arr"], linb_sum, phase=phase)
    in_maps = make_in_maps(cfg, prep, W1, b1, W2, b2, lin_W, lin_b)
    res = run_bass_kernel_spmd(
        nc, in_maps, list(range(NC)), trace=trace
    )
    LAST_EXEC_NS = res.exec_time_ns
    LAST_RESULTS = res
    return assemble_output(cfg, res.results)


def kernel(x, edge_index, edge_weight, edge_label_index,
           W1, b1, W2, b2, lin_W, lin_b):
    trace = bool(os.environ.get("KERNEL_TRACE"))
    return run(FULL, x, edge_index, edge_weight, edge_label_index,
               W1, b1, W2, b2, lin_W, lin_b, trace=trace)
